# revision 31
# baseline (speedup 1.0000x reference)
"""CIN (Compressed Interaction Network) kernel for Trainium2, 8 NeuronCores.

Problem: x (2048, 39, 16) f32; 3 CIN layers with W_i (200, 39, prev):
    z[b,o,d] = sum_{f,g} W[o,f,g] * x0[b,f,d] * h[b,g,d] + bias[o]
    h' = relu(z);  output = sum_d concat([h1,h2,h3], ch) -> (2048, 600)

Strategy (data-parallel over batch, 8 cores, 256 batch rows each):
  Per core, columns n = (b_local, d), N = 256*16 = 4096, in 16 n-tiles of 256
  (two 128-column windows each).  Matmuls run in the z^T orientation:
  psum [128 n, 200 o] accumulates over the contraction (f, g); lhsT
  (stationary) = V slices [g, n-window]; rhs (moving) = weight slices
  [g, 200].  V_f = h (.) bcast(x0[f]) is split across engines by f-slot:
    slots 0..PF-1         fp8 direct on Pool (tensor_tensor)
    slots PF..PF+YF-1     fp16 on Vector (tensor_tensor, 2x mode),
                          converted to fp8 by the Scalar engine
    slots PF+YF..38       fp16 on Vector, consumed by fp16 matmuls
  fp8 slots feed fp8e4m3 DoubleRow matmuls (two f's per instruction at
  0.5 cycles/row).  Weights are pre-scaled by 64 so fp8 stays in e4m3's
  normal range; the relu epilogue on the Scalar engine rescales by 1/64.
  Bias enters as a K=1 ones-row matmul.  L0 uses the f<=g symmetry of
  x0*x0: 780 rows (7 K-chunks) with folded weights W0+W0^T.
  h^T [n, 200] is transposed by the PE into a PSUM bank; the Vector
  engine copies it to SBUF h tiles [g, n] for the next layer's V build.
  The d-sums run on the PE as tiny K=128 matmuls against a 0/1 selector
  [128, 8], accumulated in a PSUM bank that the Scalar engine flushes
  to SBUF every 4 tiles (DMA'd to DRAM at the end).  Tiles are emitted
  pairwise-interleaved, with the next pair's L0 layer emitted between
  the current pair's L1 and L2 so the PE fills build-phase bubbles and
  h0 of the next pair is ready before its V1 build; all build/copy/
  convert instructions are window-granular (128 columns) to keep the
  cross-engine dependency chain latency short.
"""
import numpy as np

import concourse.bacc as bacc
import concourse.mybir as mybir
import concourse.tile as tile
from concourse.bass_utils import run_bass_kernel_spmd

B, F0, D = 2048, 39, 16
C = 200                      # cross size per layer
NCORES = 8
BC = B // NCORES             # 256 batch rows per core
N = BC * D                   # 4096 columns per core
NT = 256                     # n-tile width
T = N // NT                  # 16 n-tiles
BT = NT // D                 # 16 batch rows per n-tile
NW = NT // 128               # 2 matmul windows per tile
K0 = 7                       # L0 symmetric K-chunks (780 rows padded to 896)
NPAIR = F0 * (F0 + 1) // 2   # 780
PF = 8                       # f-slots built fp8 directly on Pool
YF = 14                      # f-slots built fp16 on Vector, Act-converted
XF = F0 - PF - YF            # 17 f-slots kept fp16 end to end
NF8 = PF + YF                # 26 fp8 slots
NP8 = NF8 // 2               # 13 DoubleRow pairs
NV16 = YF + XF               # 28 f-slots built on Vector (fp16)
GA, GB = 128, C - 128        # g-split (h partition split 128 + 72)
SCALE = 64.0                 # weight pre-scale (power of 2)
GRP = 4                      # tiles per output-psum flush group
F16 = mybir.dt.float16
F8 = mybir.dt.float8e4
F32 = mybir.dt.float32


def _build_nc():
    nc = bacc.Bacc(None, target_bir_lowering=False)
    mult = mybir.AluOpType.mult
    relu = mybir.ActivationFunctionType.Relu

    x0_d = nc.dram_tensor("x0", [F0, N], F16, kind="ExternalInput")
    v0_d = nc.dram_tensor("v0", [K0 * 128, N], F16, kind="ExternalInput")
    w0_d = nc.dram_tensor("w0", [128, K0 * C], F16, kind="ExternalInput")
    w1a_d = nc.dram_tensor("w1a", [GA, XF * C], F16, kind="ExternalInput")
    w1b_d = nc.dram_tensor("w1b", [GB, XF * C], F16, kind="ExternalInput")
    w2a_d = nc.dram_tensor("w2a", [GA, XF * C], F16, kind="ExternalInput")
    w2b_d = nc.dram_tensor("w2b", [GB, XF * C], F16, kind="ExternalInput")
    w18a_d = nc.dram_tensor("w18a", [GA, NF8 * C], F8, kind="ExternalInput")
    w18b_d = nc.dram_tensor("w18b", [GB, NF8 * C], F8, kind="ExternalInput")
    w28a_d = nc.dram_tensor("w28a", [GA, NF8 * C], F8, kind="ExternalInput")
    w28b_d = nc.dram_tensor("w28b", [GB, NF8 * C], F8, kind="ExternalInput")
    brow_d = nc.dram_tensor("brow", [1, 3 * C], F16, kind="ExternalInput")
    ones_d = nc.dram_tensor("ones1", [1, 128], F16, kind="ExternalInput")
    id_d = nc.dram_tensor("ident", [128, 128], F16, kind="ExternalInput")
    smat_d = nc.dram_tensor("smat", [128, BT // NW], F16, kind="ExternalInput")
    outa_d = nc.dram_tensor("outa", [GA, 3 * N // D], F32, kind="ExternalOutput")
    outb_d = nc.dram_tensor("outb", [GB, 3 * N // D], F32, kind="ExternalOutput")

    with tile.TileContext(nc) as tc:
        with (
            tc.tile_pool(name="wp", bufs=1) as wp,
            tc.tile_pool(name="bc", bufs=2) as bcp,
            tc.tile_pool(name="hs", bufs=2) as hsp,
            tc.tile_pool(name="ht", bufs=6) as htp,
            tc.tile_pool(name="va", bufs=2) as vap,
            tc.tile_pool(name="ps", bufs=3, space="PSUM") as ps,
            tc.tile_pool(name="pt", bufs=3, space="PSUM") as pt,
            tc.tile_pool(name="op", bufs=2, space="PSUM") as opp,
        ):
            # --- static state -------------------------------------------------
            w0 = wp.tile([128, K0 * C], F16)
            nc.sync.dma_start(out=w0[:], in_=w0_d[:])
            brow = wp.tile([1, 3 * C], F16)
            nc.sync.dma_start(out=brow[:], in_=brow_d[:])
            ones1 = wp.tile([1, 128], F16)
            nc.sync.dma_start(out=ones1[:], in_=ones_d[:])
            ident = wp.tile([128, 128], F16)
            nc.sync.dma_start(out=ident[:], in_=id_d[:])
            smat = wp.tile([128, BT // NW], F16)
            nc.sync.dma_start(out=smat[:], in_=smat_d[:])
            outa_s = wp.tile([GA, 3 * N // D], F32)
            outb_s = wp.tile([GB, 3 * N // D], F32)

            def emit_v0(t):
                v0t = bcp.tile([128, K0 * NT], F16, tag="v0t")
                src = (v0_d[:].rearrange("(c p) n -> p c n", p=128)
                       [:, :, t * NT:(t + 1) * NT])
                nc.sync.dma_start(
                    out=v0t[:].rearrange("p (c n) -> p c n", n=NT), in_=src)
                return v0t

            def emit_xb(t, fchunk=13):
                xb = bcp.tile([128, F0 * NT], F16, tag="xb")
                for f0 in range(0, F0, fchunk):
                    f1 = min(f0 + fchunk, F0)
                    src = (x0_d[f0:f1, t * NT:(t + 1) * NT]
                           .unsqueeze(0).broadcast_to((128, f1 - f0, NT)))
                    nc.sync.dma_start(
                        out=xb[:, f0 * NT:f1 * NT]
                        .rearrange("p (f n) -> p f n", n=NT), in_=src)
                return xb, None

            def emit_build(xbp, ha, hb):
                # Window-granular build: every instruction covers one
                # 128-column window so the downstream matmul/conv chain
                # starts after ~1-2 us instead of ~4-8 us.
                #   va8/vb8 [*, NF8*NT] f8: slots 0..PF-1 Pool-direct,
                #     slots PF..NF8-1 Act-converted from va cols 0..YF-1
                #   va/vb [*, NV16*NT] f16: col j <-> f-slot PF+j
                xb, xbr = xbp
                va = vap.tile([GA, NV16 * NT], F16, tag="va")
                vb = vap.tile([GB, NV16 * NT], F16, tag="vb")
                va8 = vap.tile([GA, NF8 * NT], F8, tag="va8")
                vb8 = vap.tile([GB, NF8 * NT], F8, tag="vb8")
                v8w = va8[:].rearrange("p (f w n) -> p f w n", f=NF8, w=NW)
                v8bw = vb8[:].rearrange("p (f w n) -> p f w n", f=NF8, w=NW)
                vaw = va[:].rearrange("p (f w n) -> p f w n", f=NV16, w=NW)
                vbw = vb[:].rearrange("p (f w n) -> p f w n", f=NV16, w=NW)
                xbw = xb[:].rearrange("p (f w n) -> p f w n", f=F0, w=NW)
                for w in range(NW):
                    nc.gpsimd.tensor_tensor(
                        out=v8w[:, 0:PF, w],
                        in0=ha[:, w].unsqueeze(1).broadcast_to((GA, PF, 128)),
                        in1=xbw[0:GA, 0:PF, w], op=mult)
                    nc.gpsimd.tensor_tensor(
                        out=v8bw[:, 0:PF, w],
                        in0=hb[:, w].unsqueeze(1).broadcast_to((GB, PF, 128)),
                        in1=xbw[0:GB, 0:PF, w], op=mult)
                    nc.vector.tensor_tensor(
                        out=vaw[:, 0:YF, w],
                        in0=ha[:, w].unsqueeze(1).broadcast_to((GA, YF, 128)),
                        in1=xbw[0:GA, PF:PF + YF, w], op=mult)
                    nc.vector.tensor_tensor(
                        out=vbw[:, 0:YF, w],
                        in0=hb[:, w].unsqueeze(1).broadcast_to((GB, YF, 128)),
                        in1=xbw[0:GB, PF:PF + YF, w], op=mult)
                    nc.scalar.copy(out=v8w[:, PF:NF8, w],
                                   in_=vaw[:, 0:YF, w])
                    nc.scalar.copy(out=v8bw[:, PF:NF8, w],
                                   in_=vbw[:, 0:YF, w])
                    nc.vector.tensor_tensor(
                        out=vaw[:, YF:NV16, w],
                        in0=ha[:, w].unsqueeze(1).broadcast_to((GA, XF, 128)),
                        in1=xbw[0:GA, PF + YF:F0, w], op=mult)
                    nc.vector.tensor_tensor(
                        out=vbw[:, YF:NV16, w],
                        in0=hb[:, w].unsqueeze(1).broadcast_to((GB, XF, 128)),
                        in1=xbw[0:GB, PF + YF:F0, w], op=mult)
                return va, vb, va8, vb8

            def emit_l0_win(v0t, w, l):
                pz = ps.tile([128, C], F32, tag="pz")
                nc.tensor.matmul(pz[:], ones1[:],
                                 brow[:, l * C:(l + 1) * C],
                                 start=True, stop=False)
                v3 = v0t[:].rearrange("p (c n) -> p c n", n=NT)
                for c in range(K0):
                    nc.tensor.matmul(pz[:], v3[:, c, w * 128:(w + 1) * 128],
                                     w0[:, c * C:(c + 1) * C],
                                     start=False, stop=(c == K0 - 1))
                return pz

            def emit_l12_win(vs, wa, wb, w8a, w8b, w, l):
                va, vb, va8, vb8 = vs
                pz = ps.tile([128, C], F32, tag="pz")
                ws = slice(w * 128, (w + 1) * 128)
                nc.tensor.matmul(pz[:], ones1[:],
                                 brow[:, l * C:(l + 1) * C],
                                 start=True, stop=False)
                v3a = va[:].rearrange("p (f n) -> p f n", n=NT)
                v3b = vb[:].rearrange("p (f n) -> p f n", n=NT)
                for j in range(XF):
                    # fp16 f-slot PF+YF+j lives at va col YF+j
                    p = YF + j
                    nc.tensor.matmul(pz[:], v3a[:, p, ws],
                                     wa[:, j * C:(j + 1) * C],
                                     start=False, stop=False)
                    nc.tensor.matmul(pz[:], v3b[:, p, ws],
                                     wb[:, j * C:(j + 1) * C],
                                     start=False, stop=False)
                p3a = va8[:].rearrange("p (f n) -> p f n", n=NT)
                p3b = vb8[:].rearrange("p (f n) -> p f n", n=NT)
                w4a = w8a[:].rearrange("p (j t o) -> p j t o", t=2, o=C)
                w4b = w8b[:].rearrange("p (j t o) -> p j t o", t=2, o=C)
                for j in range(NP8):
                    nc.tensor.matmul(pz[:], p3a[:, 2 * j:2 * j + 2, ws],
                                     w4a[:, j], start=False, stop=False,
                                     perf_mode=mybir.MatmulPerfMode.DoubleRow)
                    nc.tensor.matmul(pz[:], p3b[:, 2 * j:2 * j + 2, ws],
                                     w4b[:, j], start=False, stop=(j == NP8 - 1),
                                     perf_mode=mybir.MatmulPerfMode.DoubleRow)
                return pz

            def emit_epi(pz, w, l, t, pout, hps=None, ha=None, hb=None,
                         hsb=None, do_copies=True):
                # relu (+1/64 rescale) -> h^T [128, 200]; d-sum on the PE;
                # for l<2: transpose into a PSUM bank, copy to SBUF h tiles
                hT = htp.tile([128, C], F16, tag="hT")
                nc.scalar.activation(hT[:], pz[:], relu, scale=1.0 / SCALE)
                if l < 2:
                    ws = slice(2 * w * 128, (2 * w + 1) * 128)
                    ws2 = slice((2 * w + 1) * 128, (2 * w + 2) * 128)
                    nc.tensor.transpose(hps[:, ws], hT[:, 0:GA], ident[:])
                    nc.tensor.transpose(hps[0:GB, ws2], hT[:, GA:C], ident[:])
                    if do_copies:
                        wf = slice(2 * w * 128, (2 * w + 2) * 128)
                        nc.vector.tensor_copy(out=hsb[:, wf],
                                              in_=hps[:, wf])
                off = l * 64 + (t % GRP) * BT + w * (BT // NW)
                nc.tensor.matmul(pout[:, off:off + BT // NW],
                                 hT[:, 0:GA], smat[:], start=True, stop=True)
                nc.tensor.matmul(pout[0:GB, 192 + off:192 + off + BT // NW],
                                 hT[:, GA:C], smat[:], start=True, stop=True)

            # --- weights ------------------------------------------------------
            v00, v01 = emit_v0(0), emit_v0(1)
            xb0, xb1 = emit_xb(0), emit_xb(1)
            w1a = wp.tile([GA, XF * C], F16)
            nc.sync.dma_start(out=w1a[:], in_=w1a_d[:])
            w1b = wp.tile([GB, XF * C], F16)
            nc.sync.dma_start(out=w1b[:], in_=w1b_d[:])
            w18a = wp.tile([GA, NF8 * C], F8)
            nc.sync.dma_start(out=w18a[:], in_=w18a_d[:])
            w18b = wp.tile([GB, NF8 * C], F8)
            nc.sync.dma_start(out=w18b[:], in_=w18b_d[:])
            w2a = wp.tile([GA, XF * C], F16)
            nc.sync.dma_start(out=w2a[:], in_=w2a_d[:])
            w2b = wp.tile([GB, XF * C], F16)
            nc.sync.dma_start(out=w2b[:], in_=w2b_d[:])
            w28a = wp.tile([GA, NF8 * C], F8)
            nc.sync.dma_start(out=w28a[:], in_=w28a_d[:])
            w28b = wp.tile([GB, NF8 * C], F8)
            nc.sync.dma_start(out=w28b[:], in_=w28b_d[:])

            # --- pipeline: rotated so L0(p+1) overlaps V2(p)/L2(p) -----------
            def alloc_h(tp, l):
                hts = []
                for k in range(2):
                    hps = pt.tile([128, 2 * NT], F16, tag="hps",
                                  name=f"hps{l}_{tp}_{k}")
                    hsb = hsp.tile([GA, 2 * NT], F16, tag=f"h{l}s{k}",
                                   name=f"h{l}s{k}_{tp}")
                    # window-major: [w0-a 0:128, w0-b 128:256, w1-a ...]
                    ha = hsb[:].rearrange("p (w t n) -> p w t n",
                                          w=NW, t=2)[:, :, 0, :]
                    hb = (hsb[0:GB].rearrange("p (w t n) -> p w t n",
                                              w=NW, t=2)[:, :, 1, :])
                    hts.append((hps, ha, hb, hsb))
                return hts

            def emit_l0_pair(tp, v0s, pout, do_copies=True, copy_eng=None):
                hts = alloc_h(tp, 0)
                for w in range(NW):
                    for k in range(2):
                        pz = emit_l0_win(v0s[k], w, 0)
                        emit_epi(pz, w, 0, tp + k, pout, *hts[k],
                                 do_copies=do_copies)
                    if copy_eng is not None:
                        for k in range(2):
                            emit_h_copies(hts[k], eng=copy_eng, w=w)
                return hts

            def emit_h_copies(hts_k, eng="dve", w=None):
                hps, ha, hb, hsb = hts_k
                sl = (slice(0, 2 * NT) if w is None
                      else slice(2 * w * 128, (2 * w + 2) * 128))
                if eng == "dve":
                    nc.vector.tensor_copy(out=hsb[:, sl], in_=hps[:, sl])
                else:
                    nc.scalar.copy(out=hsb[:, sl], in_=hps[:, sl])

            # prologue: L0 of pair 0
            pout = opp.tile([128, 2 * 192], F32, tag="pout")
            h0s = emit_l0_pair(0, (v00, v01), pout)
            xbs = (xb0, xb1)

            for tp in range(0, T, 2):
                t0, t1 = tp, tp + 1
                # prefetch pair p+1 inputs early
                if tp + 2 < T:
                    v0n = (emit_v0(tp + 2), emit_v0(tp + 3))
                    xbn = (emit_xb(tp + 2), emit_xb(tp + 3))
                # V1 builds + L1 windows
                h1s = alloc_h(tp, 1)
                vss = [emit_build(xbs[k], h0s[k][1], h0s[k][2])
                       for k in range(2)]
                for w in range(NW):
                    ks = ((0, t0), (1, t1)) if w == 0 else ((1, t1), (0, t0))
                    for k, t in ks:
                        pz = emit_l12_win(vss[k], w1a, w1b, w18a, w18b, w, 1)
                        emit_epi(pz, w, 1, t, pout, *h1s[k],
                                 do_copies=False)
                    for k in range(2):
                        emit_h_copies(h1s[k], eng="act", w=w)
                # L0 of pair p+1 runs on the PE while V2 builds; its h
                # copies interleave between the V2-A and V2-B build chunks
                pout_cur = pout
                h0n = None
                if tp + 2 < T:
                    if (tp + 2) % GRP == 0:
                        pout = opp.tile([128, 2 * 192], F32, tag="pout")
                    h0n = emit_l0_pair(tp + 2, v0n, pout, do_copies=False)
                    for k in range(2):
                        emit_h_copies(h0n[k], eng="act")
                vss = [emit_build(xbs[k], h1s[k][1], h1s[k][2])
                       for k in range(2)]
                if tp + 2 < T:
                    h0s = h0n
                    xbs = xbn
                # L2 windows
                for w in range(NW):
                    for k, t in ((0, t0), (1, t1)):
                        pz = emit_l12_win(vss[k], w2a, w2b, w28a, w28b, w, 2)
                        emit_epi(pz, w, 2, t, pout_cur)
                if tp % GRP == GRP - 2:
                    g = tp // GRP
                    nc.scalar.copy(out=outa_s[:, g * 192:(g + 1) * 192],
                                   in_=pout_cur[:, 0:192])
                    nc.scalar.copy(out=outb_s[:, g * 192:(g + 1) * 192],
                                   in_=pout_cur[0:GB, 192:384])

            nc.sync.dma_start(out=outa_d[:], in_=outa_s[:])
            nc.sync.dma_start(out=outb_d[:], in_=outb_s[:])

    nc.compile()
    return nc


_NC_CACHE = None


def _get_nc():
    global _NC_CACHE
    if _NC_CACHE is None:
        _NC_CACHE = _build_nc()
    return _NC_CACHE


def _q8(x):
    import ml_dtypes
    return np.asarray(x, np.float32).astype(ml_dtypes.float8_e4m3fn)


def _prep_weights(W0, b0, W1, b1, W2, b2):
    # L0: symmetric fold.  W0eff[o, (f,g)] = W0[o,f,g]+W0[o,g,f] (f<g),
    # W0[o,f,f] on the diagonal; pairs in triu order, padded 780 -> 896.
    W0 = np.asarray(W0, np.float32)
    iu0, iu1 = np.triu_indices(F0)
    Wsym = W0 + W0.transpose(0, 2, 1)
    dd = np.arange(F0)
    Wsym[:, dd, dd] = W0[:, dd, dd]
    w0f = np.zeros((K0 * 128, C), np.float32)
    w0f[0:NPAIR] = Wsym[:, iu0, iu1].T * SCALE
    w0 = np.ascontiguousarray(
        w0f.reshape(K0, 128, C).transpose(1, 0, 2).reshape(128, K0 * C)
    ).astype(np.float16)

    def lay(W):
        # Wt[g, f, o] = SCALE*W[o, f, g]; slot order = f order
        Wt = np.asarray(W, np.float32).transpose(2, 1, 0) * SCALE
        wa = np.ascontiguousarray(Wt[0:GA, NF8:].reshape(GA, XF * C)
                                  ).astype(np.float16)
        wb = np.ascontiguousarray(Wt[GA:C, NF8:].reshape(GB, XF * C)
                                  ).astype(np.float16)
        w8a = _q8(np.ascontiguousarray(Wt[0:GA, 0:NF8].reshape(GA, NF8 * C)))
        w8b = _q8(np.ascontiguousarray(Wt[GA:C, 0:NF8].reshape(GB, NF8 * C)))
        return wa, wb, w8a, w8b

    w1a, w1b, w18a, w18b = lay(W1)
    w2a, w2b, w28a, w28b = lay(W2)
    brow = np.zeros((1, 3 * C), np.float16)
    for l, b in enumerate((b0, b1, b2)):
        brow[0, l * C:(l + 1) * C] = (np.asarray(b, np.float32) * SCALE
                                      ).astype(np.float16)
    smat = np.zeros((128, BT // NW), np.float16)
    smat[np.arange(128), np.arange(128) // D] = 1.0
    return {
        "w0": w0, "w1a": w1a, "w1b": w1b, "w2a": w2a, "w2b": w2b,
        "w18a": w18a, "w18b": w18b, "w28a": w28a, "w28b": w28b,
        "brow": brow,
        "ones1": np.ones((1, 128), np.float16),
        "ident": np.eye(128, dtype=np.float16),
        "smat": smat,
    }


def kernel(x, W0, b0, W1, b1, W2, b2):
    x = np.asarray(x)
    assert x.shape == (B, F0, D), x.shape
    nc = _get_nc()
    shared = _prep_weights(W0, b0, W1, b1, W2, b2)
    iu0, iu1 = np.triu_indices(F0)

    in_maps = []
    for c in range(NCORES):
        xc = x[c * BC:(c + 1) * BC]                      # [256, 39, 16]
        x0c = np.ascontiguousarray(
            xc.transpose(1, 0, 2).reshape(F0, N)).astype(np.float16)
        x0f32 = x0c.astype(np.float32)
        v0 = np.zeros((K0 * 128, N), np.float16)
        v0[0:NPAIR] = (x0f32[iu0] * x0f32[iu1]).astype(np.float16)
        in_maps.append({"x0": x0c, "v0": v0, **shared})

    res = run_bass_kernel_spmd(nc, in_maps, list(range(NCORES)))

    out = np.empty((B, 3 * C), dtype=np.float32)
    for c in range(NCORES):
        # outa cols: g*192 + l*64 + tl*16 + b16, tiles t = g*4+tl
        oa = res.results[c]["outa"]                      # [128, 768]
        ob = res.results[c]["outb"]                      # [72, 768]
        oa = oa.reshape(GA, 4, 3, GRP, BT).transpose(1, 3, 4, 2, 0)
        ob = ob.reshape(GB, 4, 3, GRP, BT).transpose(1, 3, 4, 2, 0)
        oc = np.concatenate(
            [oa.reshape(BC, 3, GA), ob.reshape(BC, 3, GB)], axis=2)
        out[c * BC:(c + 1) * BC] = oc.reshape(BC, 3 * C)
    return out


# revision 32
# speedup vs baseline: 1.1000x; 1.1000x over previous
"""CIN (Compressed Interaction Network) kernel for Trainium2, 8 NeuronCores.

Problem: x (2048, 39, 16) f32; 3 CIN layers with W_i (200, 39, prev):
    z[b,o,d] = sum_{f,g} W[o,f,g] * x0[b,f,d] * h[b,g,d] + bias[o]
    h' = relu(z);  output = sum_d concat([h1,h2,h3], ch) -> (2048, 600)

Strategy (data-parallel over batch, 8 cores, 256 batch rows each):
  Per core, columns n = (b_local, d), N = 256*16 = 4096, in 16 n-tiles of 256
  (two 128-column windows each).  Matmuls run in the z^T orientation:
  psum [128 n, 200 o] accumulates over the contraction (f, g); lhsT
  (stationary) = V slices [g, n-window]; rhs (moving) = weight slices
  [g, 200].  V_f = h (.) bcast(x0[f]) is split across engines by f-slot:
    slots 0..PF-1         fp8 direct on Pool (tensor_tensor)
    slots PF..PF+YF-1     fp16 on Vector (tensor_tensor, 2x mode),
                          converted to fp8 by the Scalar engine
    slots PF+YF..38       fp16 on Vector, consumed by fp16 matmuls
  fp8 slots feed fp8e4m3 DoubleRow matmuls (two f's per instruction at
  0.5 cycles/row).  Weights are pre-scaled by 64 so fp8 stays in e4m3's
  normal range; the relu epilogue on the Scalar engine rescales by 1/64.
  Bias enters as a K=1 ones-row matmul.  L0 uses the f<=g symmetry of
  x0*x0: 780 rows (7 K-chunks) with folded weights W0+W0^T.
  h^T [n, 200] is transposed by the PE into a PSUM bank; the Vector
  engine copies it to SBUF h tiles [g, n] for the next layer's V build.
  The d-sums run on the PE as tiny K=128 matmuls against a 0/1 selector
  [128, 8], accumulated in a PSUM bank that the Scalar engine flushes
  to SBUF every 4 tiles (DMA'd to DRAM at the end).  Tiles are emitted
  pairwise-interleaved, with the next pair's L0 layer emitted between
  the current pair's L1 and L2 so the PE fills build-phase bubbles and
  h0 of the next pair is ready before its V1 build; all build/copy/
  convert instructions are window-granular (128 columns) to keep the
  cross-engine dependency chain latency short.
"""
import numpy as np

import concourse.bacc as bacc
import concourse.mybir as mybir
import concourse.tile as tile
from concourse.bass_utils import run_bass_kernel_spmd

B, F0, D = 2048, 39, 16
C = 200                      # cross size per layer
NCORES = 8
BC = B // NCORES             # 256 batch rows per core
N = BC * D                   # 4096 columns per core
NT = 256                     # n-tile width
T = N // NT                  # 16 n-tiles
BT = NT // D                 # 16 batch rows per n-tile
NW = NT // 128               # 2 matmul windows per tile
K0 = 7                       # L0 symmetric K-chunks (780 rows padded to 896)
NPAIR = F0 * (F0 + 1) // 2   # 780
PF = 8                       # f-slots built fp8 directly on Pool
YF = 14                      # f-slots built fp16 on Vector, Act-converted
XF = F0 - PF - YF            # 17 f-slots kept fp16 end to end
NF8 = PF + YF                # 26 fp8 slots
NP8 = NF8 // 2               # 13 DoubleRow pairs
NV16 = YF + XF               # 28 f-slots built on Vector (fp16)
GA, GB = 128, C - 128        # g-split (h partition split 128 + 72)
SCALE = 64.0                 # weight pre-scale (power of 2)
GRP = 4                      # tiles per output-psum flush group
F16 = mybir.dt.float16
F8 = mybir.dt.float8e4
F32 = mybir.dt.float32


def _build_nc():
    nc = bacc.Bacc(None, target_bir_lowering=False)
    mult = mybir.AluOpType.mult
    relu = mybir.ActivationFunctionType.Relu

    x0_d = nc.dram_tensor("x0", [F0, N], F16, kind="ExternalInput")
    v0_d = nc.dram_tensor("v0", [K0 * 128, N], F16, kind="ExternalInput")
    w0_d = nc.dram_tensor("w0", [128, K0 * C], F16, kind="ExternalInput")
    w1a_d = nc.dram_tensor("w1a", [GA, XF * C], F16, kind="ExternalInput")
    w1b_d = nc.dram_tensor("w1b", [GB, XF * C], F16, kind="ExternalInput")
    w2a_d = nc.dram_tensor("w2a", [GA, XF * C], F16, kind="ExternalInput")
    w2b_d = nc.dram_tensor("w2b", [GB, XF * C], F16, kind="ExternalInput")
    w18a_d = nc.dram_tensor("w18a", [GA, NF8 * C], F8, kind="ExternalInput")
    w18b_d = nc.dram_tensor("w18b", [GB, NF8 * C], F8, kind="ExternalInput")
    w28a_d = nc.dram_tensor("w28a", [GA, NF8 * C], F8, kind="ExternalInput")
    w28b_d = nc.dram_tensor("w28b", [GB, NF8 * C], F8, kind="ExternalInput")
    brow_d = nc.dram_tensor("brow", [1, 3 * C], F16, kind="ExternalInput")
    ones_d = nc.dram_tensor("ones1", [1, 128], F16, kind="ExternalInput")
    id_d = nc.dram_tensor("ident", [128, 128], F16, kind="ExternalInput")
    smat_d = nc.dram_tensor("smat", [128, BT // NW], F16, kind="ExternalInput")
    outa_d = nc.dram_tensor("outa", [GA, 3 * N // D], F32, kind="ExternalOutput")
    outb_d = nc.dram_tensor("outb", [GB, 3 * N // D], F32, kind="ExternalOutput")

    with tile.TileContext(nc) as tc:
        with (
            tc.tile_pool(name="wp", bufs=1) as wp,
            tc.tile_pool(name="bc", bufs=2) as bcp,
            tc.tile_pool(name="hs", bufs=2) as hsp,
            tc.tile_pool(name="ht", bufs=6) as htp,
            tc.tile_pool(name="va", bufs=2) as vap,
            tc.tile_pool(name="ps", bufs=3, space="PSUM") as ps,
            tc.tile_pool(name="pt", bufs=3, space="PSUM") as pt,
            tc.tile_pool(name="op", bufs=2, space="PSUM") as opp,
        ):
            # --- static state -------------------------------------------------
            w0 = wp.tile([128, K0 * C], F16)
            nc.sync.dma_start(out=w0[:], in_=w0_d[:])
            brow = wp.tile([1, 3 * C], F16)
            nc.sync.dma_start(out=brow[:], in_=brow_d[:])
            ones1 = wp.tile([1, 128], F16)
            nc.sync.dma_start(out=ones1[:], in_=ones_d[:])
            ident = wp.tile([128, 128], F16)
            nc.sync.dma_start(out=ident[:], in_=id_d[:])
            smat = wp.tile([128, BT // NW], F16)
            nc.sync.dma_start(out=smat[:], in_=smat_d[:])
            outa_s = wp.tile([GA, 3 * N // D], F32)
            outb_s = wp.tile([GB, 3 * N // D], F32)

            def emit_v0(t):
                v0t = bcp.tile([128, K0 * NT], F16, tag="v0t")
                src = (v0_d[:].rearrange("(c p) n -> p c n", p=128)
                       [:, :, t * NT:(t + 1) * NT])
                nc.sync.dma_start(
                    out=v0t[:].rearrange("p (c n) -> p c n", n=NT), in_=src)
                return v0t

            def emit_xb(t, fchunk=13):
                xb = bcp.tile([128, F0 * NT], F16, tag="xb")
                for f0 in range(0, F0, fchunk):
                    f1 = min(f0 + fchunk, F0)
                    src = (x0_d[f0:f1, t * NT:(t + 1) * NT]
                           .unsqueeze(0).broadcast_to((128, f1 - f0, NT)))
                    nc.sync.dma_start(
                        out=xb[:, f0 * NT:f1 * NT]
                        .rearrange("p (f n) -> p f n", n=NT), in_=src)
                return xb, None

            def emit_build(xbp, ha, hb):
                # Window-granular build: every instruction covers one
                # 128-column window so the downstream matmul/conv chain
                # starts after ~1-2 us instead of ~4-8 us.
                #   va8/vb8 [*, NF8*NT] f8: slots 0..PF-1 Pool-direct,
                #     slots PF..NF8-1 Act-converted from va cols 0..YF-1
                #   va/vb [*, NV16*NT] f16: col j <-> f-slot PF+j
                xb, xbr = xbp
                va = vap.tile([GA, NV16 * NT], F16, tag="va")
                vb = vap.tile([GB, NV16 * NT], F16, tag="vb")
                va8 = vap.tile([GA, NF8 * NT], F8, tag="va8")
                vb8 = vap.tile([GB, NF8 * NT], F8, tag="vb8")
                v8w = va8[:].rearrange("p (f w n) -> p f w n", f=NF8, w=NW)
                v8bw = vb8[:].rearrange("p (f w n) -> p f w n", f=NF8, w=NW)
                vaw = va[:].rearrange("p (f w n) -> p f w n", f=NV16, w=NW)
                vbw = vb[:].rearrange("p (f w n) -> p f w n", f=NV16, w=NW)
                xbw = xb[:].rearrange("p (f w n) -> p f w n", f=F0, w=NW)
                for w in range(NW):
                    nc.gpsimd.tensor_tensor(
                        out=v8w[:, 0:PF, w],
                        in0=ha[:, w].unsqueeze(1).broadcast_to((GA, PF, 128)),
                        in1=xbw[0:GA, 0:PF, w], op=mult)
                    nc.gpsimd.tensor_tensor(
                        out=v8bw[:, 0:PF, w],
                        in0=hb[:, w].unsqueeze(1).broadcast_to((GB, PF, 128)),
                        in1=xbw[0:GB, 0:PF, w], op=mult)
                    nc.vector.tensor_tensor(
                        out=vaw[:, 0:YF, w],
                        in0=ha[:, w].unsqueeze(1).broadcast_to((GA, YF, 128)),
                        in1=xbw[0:GA, PF:PF + YF, w], op=mult)
                    nc.vector.tensor_tensor(
                        out=vbw[:, 0:YF, w],
                        in0=hb[:, w].unsqueeze(1).broadcast_to((GB, YF, 128)),
                        in1=xbw[0:GB, PF:PF + YF, w], op=mult)
                    nc.scalar.copy(out=v8w[:, PF:NF8, w],
                                   in_=vaw[:, 0:YF, w])
                    nc.scalar.copy(out=v8bw[:, PF:NF8, w],
                                   in_=vbw[:, 0:YF, w])
                    nc.vector.tensor_tensor(
                        out=vaw[:, YF:NV16, w],
                        in0=ha[:, w].unsqueeze(1).broadcast_to((GA, XF, 128)),
                        in1=xbw[0:GA, PF + YF:F0, w], op=mult)
                    nc.vector.tensor_tensor(
                        out=vbw[:, YF:NV16, w],
                        in0=hb[:, w].unsqueeze(1).broadcast_to((GB, XF, 128)),
                        in1=xbw[0:GB, PF + YF:F0, w], op=mult)
                return va, vb, va8, vb8

            def emit_l0_win(v0t, w, l):
                pz = ps.tile([128, C], F32, tag="pz")
                nc.tensor.matmul(pz[:], ones1[:],
                                 brow[:, l * C:(l + 1) * C],
                                 start=True, stop=False)
                v3 = v0t[:].rearrange("p (c n) -> p c n", n=NT)
                for c in range(K0):
                    nc.tensor.matmul(pz[:], v3[:, c, w * 128:(w + 1) * 128],
                                     w0[:, c * C:(c + 1) * C],
                                     start=False, stop=(c == K0 - 1))
                return pz

            def emit_l12_win(vs, wa, wb, w8a, w8b, w, l):
                va, vb, va8, vb8 = vs
                pz = ps.tile([128, C], F32, tag="pz")
                ws = slice(w * 128, (w + 1) * 128)
                nc.tensor.matmul(pz[:], ones1[:],
                                 brow[:, l * C:(l + 1) * C],
                                 start=True, stop=False)
                v3a = va[:].rearrange("p (f n) -> p f n", n=NT)
                v3b = vb[:].rearrange("p (f n) -> p f n", n=NT)
                for j in range(XF):
                    # fp16 f-slot PF+YF+j lives at va col YF+j
                    p = YF + j
                    nc.tensor.matmul(pz[:], v3a[:, p, ws],
                                     wa[:, j * C:(j + 1) * C],
                                     start=False, stop=False)
                    nc.tensor.matmul(pz[:], v3b[:, p, ws],
                                     wb[:, j * C:(j + 1) * C],
                                     start=False, stop=False)
                p3a = va8[:].rearrange("p (f n) -> p f n", n=NT)
                p3b = vb8[:].rearrange("p (f n) -> p f n", n=NT)
                w4a = w8a[:].rearrange("p (j t o) -> p j t o", t=2, o=C)
                w4b = w8b[:].rearrange("p (j t o) -> p j t o", t=2, o=C)
                for j in range(NP8):
                    nc.tensor.matmul(pz[:], p3a[:, 2 * j:2 * j + 2, ws],
                                     w4a[:, j], start=False, stop=False,
                                     perf_mode=mybir.MatmulPerfMode.DoubleRow)
                    nc.tensor.matmul(pz[:], p3b[:, 2 * j:2 * j + 2, ws],
                                     w4b[:, j], start=False, stop=(j == NP8 - 1),
                                     perf_mode=mybir.MatmulPerfMode.DoubleRow)
                return pz

            def emit_epi(pz, w, l, t, pout, hps=None, ha=None, hb=None,
                         hsb=None, do_copies=True):
                # relu (+1/64 rescale) -> h^T [128, 200]; d-sum on the PE;
                # for l<2: transpose into a PSUM bank, copy to SBUF h tiles
                hT = htp.tile([128, C], F16, tag="hT")
                nc.scalar.activation(hT[:], pz[:], relu, scale=1.0 / SCALE)
                if l < 2:
                    ws = slice(2 * w * 128, (2 * w + 1) * 128)
                    ws2 = slice((2 * w + 1) * 128, (2 * w + 2) * 128)
                    nc.tensor.transpose(hps[:, ws], hT[:, 0:GA], ident[:])
                    nc.tensor.transpose(hps[0:GB, ws2], hT[:, GA:C], ident[:])
                    if do_copies:
                        wf = slice(2 * w * 128, (2 * w + 2) * 128)
                        nc.vector.tensor_copy(out=hsb[:, wf],
                                              in_=hps[:, wf])
                off = l * 64 + (t % GRP) * BT + w * (BT // NW)
                nc.tensor.matmul(pout[:, off:off + BT // NW],
                                 hT[:, 0:GA], smat[:], start=True, stop=True)
                nc.tensor.matmul(pout[0:GB, 192 + off:192 + off + BT // NW],
                                 hT[:, GA:C], smat[:], start=True, stop=True)

            # --- weights ------------------------------------------------------
            v00, v01 = emit_v0(0), emit_v0(1)
            xb0, xb1 = emit_xb(0), emit_xb(1)
            w1a = wp.tile([GA, XF * C], F16)
            nc.sync.dma_start(out=w1a[:], in_=w1a_d[:])
            w1b = wp.tile([GB, XF * C], F16)
            nc.sync.dma_start(out=w1b[:], in_=w1b_d[:])
            w18a = wp.tile([GA, NF8 * C], F8)
            nc.sync.dma_start(out=w18a[:], in_=w18a_d[:])
            w18b = wp.tile([GB, NF8 * C], F8)
            nc.sync.dma_start(out=w18b[:], in_=w18b_d[:])
            w2a = wp.tile([GA, XF * C], F16)
            nc.sync.dma_start(out=w2a[:], in_=w2a_d[:])
            w2b = wp.tile([GB, XF * C], F16)
            nc.sync.dma_start(out=w2b[:], in_=w2b_d[:])
            w28a = wp.tile([GA, NF8 * C], F8)
            nc.sync.dma_start(out=w28a[:], in_=w28a_d[:])
            w28b = wp.tile([GB, NF8 * C], F8)
            nc.sync.dma_start(out=w28b[:], in_=w28b_d[:])

            # --- pipeline: rotated so L0(p+1) overlaps V2(p)/L2(p) -----------
            def alloc_h(tp, l):
                hts = []
                for k in range(2):
                    hps = pt.tile([128, 2 * NT], F16, tag="hps",
                                  name=f"hps{l}_{tp}_{k}")
                    hsb = hsp.tile([GA, 2 * NT], F16, tag=f"h{l}s{k}",
                                   name=f"h{l}s{k}_{tp}")
                    # window-major: [w0-a 0:128, w0-b 128:256, w1-a ...]
                    ha = hsb[:].rearrange("p (w t n) -> p w t n",
                                          w=NW, t=2)[:, :, 0, :]
                    hb = (hsb[0:GB].rearrange("p (w t n) -> p w t n",
                                              w=NW, t=2)[:, :, 1, :])
                    hts.append((hps, ha, hb, hsb))
                return hts

            def emit_l0_pair(tp, v0s, pout, do_copies=True, copy_eng=None):
                hts = alloc_h(tp, 0)
                for w in range(NW):
                    for k in range(2):
                        pz = emit_l0_win(v0s[k], w, 0)
                        emit_epi(pz, w, 0, tp + k, pout, *hts[k],
                                 do_copies=do_copies)
                    if copy_eng is not None:
                        for k in range(2):
                            emit_h_copies(hts[k], eng=copy_eng, w=w)
                return hts

            def emit_h_copies(hts_k, eng="dve", w=None):
                hps, ha, hb, hsb = hts_k
                sl = (slice(0, 2 * NT) if w is None
                      else slice(2 * w * 128, (2 * w + 2) * 128))
                if eng == "dve":
                    nc.vector.tensor_copy(out=hsb[:, sl], in_=hps[:, sl])
                else:
                    nc.scalar.copy(out=hsb[:, sl], in_=hps[:, sl])

            # prologue: L0 of pair 0
            pout = opp.tile([128, 2 * 192], F32, tag="pout")
            h0s = emit_l0_pair(0, (v00, v01), pout)
            xbs = (xb0, xb1)

            for tp in range(0, T, 2):
                t0, t1 = tp, tp + 1
                # prefetch pair p+1 inputs early
                if tp + 2 < T:
                    v0n = (emit_v0(tp + 2), emit_v0(tp + 3))
                    xbn = (emit_xb(tp + 2), emit_xb(tp + 3))
                # V1 builds + L1 windows
                h1s = alloc_h(tp, 1)
                vss = [emit_build(xbs[k], h0s[k][1], h0s[k][2])
                       for k in range(2)]
                for w in range(NW):
                    for k, t in ((0, t0), (1, t1)):
                        pz = emit_l12_win(vss[k], w1a, w1b, w18a, w18b, w, 1)
                        emit_epi(pz, w, 1, t, pout, *h1s[k],
                                 do_copies=False)
                    for k in range(2):
                        emit_h_copies(h1s[k], eng="act", w=w)
                # L0 of pair p+1 runs on the PE while V2 builds; its h
                # copies interleave between the V2-A and V2-B build chunks
                pout_cur = pout
                h0n = None
                if tp + 2 < T:
                    if (tp + 2) % GRP == 0:
                        pout = opp.tile([128, 2 * 192], F32, tag="pout")
                    h0n = emit_l0_pair(tp + 2, v0n, pout, do_copies=False)
                    for k in range(2):
                        emit_h_copies(h0n[k], eng="act")
                vss = [emit_build(xbs[k], h1s[k][1], h1s[k][2])
                       for k in range(2)]
                if tp + 2 < T:
                    h0s = h0n
                    xbs = xbn
                # L2 windows
                for w in range(NW):
                    for k, t in ((0, t0), (1, t1)):
                        pz = emit_l12_win(vss[k], w2a, w2b, w28a, w28b, w, 2)
                        emit_epi(pz, w, 2, t, pout_cur)
                if tp % GRP == GRP - 2:
                    g = tp // GRP
                    nc.scalar.copy(out=outa_s[:, g * 192:(g + 1) * 192],
                                   in_=pout_cur[:, 0:192])
                    nc.scalar.copy(out=outb_s[:, g * 192:(g + 1) * 192],
                                   in_=pout_cur[0:GB, 192:384])

            nc.sync.dma_start(out=outa_d[:], in_=outa_s[:])
            nc.sync.dma_start(out=outb_d[:], in_=outb_s[:])

    nc.compile()
    return nc


_NC_CACHE = None


def _get_nc():
    global _NC_CACHE
    if _NC_CACHE is None:
        _NC_CACHE = _build_nc()
    return _NC_CACHE


def _q8(x):
    import ml_dtypes
    return np.asarray(x, np.float32).astype(ml_dtypes.float8_e4m3fn)


def _prep_weights(W0, b0, W1, b1, W2, b2):
    # L0: symmetric fold.  W0eff[o, (f,g)] = W0[o,f,g]+W0[o,g,f] (f<g),
    # W0[o,f,f] on the diagonal; pairs in triu order, padded 780 -> 896.
    W0 = np.asarray(W0, np.float32)
    iu0, iu1 = np.triu_indices(F0)
    Wsym = W0 + W0.transpose(0, 2, 1)
    dd = np.arange(F0)
    Wsym[:, dd, dd] = W0[:, dd, dd]
    w0f = np.zeros((K0 * 128, C), np.float32)
    w0f[0:NPAIR] = Wsym[:, iu0, iu1].T * SCALE
    w0 = np.ascontiguousarray(
        w0f.reshape(K0, 128, C).transpose(1, 0, 2).reshape(128, K0 * C)
    ).astype(np.float16)

    def lay(W):
        # Wt[g, f, o] = SCALE*W[o, f, g]; slot order = f order
        Wt = np.asarray(W, np.float32).transpose(2, 1, 0) * SCALE
        wa = np.ascontiguousarray(Wt[0:GA, NF8:].reshape(GA, XF * C)
                                  ).astype(np.float16)
        wb = np.ascontiguousarray(Wt[GA:C, NF8:].reshape(GB, XF * C)
                                  ).astype(np.float16)
        w8a = _q8(np.ascontiguousarray(Wt[0:GA, 0:NF8].reshape(GA, NF8 * C)))
        w8b = _q8(np.ascontiguousarray(Wt[GA:C, 0:NF8].reshape(GB, NF8 * C)))
        return wa, wb, w8a, w8b

    w1a, w1b, w18a, w18b = lay(W1)
    w2a, w2b, w28a, w28b = lay(W2)
    brow = np.zeros((1, 3 * C), np.float16)
    for l, b in enumerate((b0, b1, b2)):
        brow[0, l * C:(l + 1) * C] = (np.asarray(b, np.float32) * SCALE
                                      ).astype(np.float16)
    smat = np.zeros((128, BT // NW), np.float16)
    smat[np.arange(128), np.arange(128) // D] = 1.0
    return {
        "w0": w0, "w1a": w1a, "w1b": w1b, "w2a": w2a, "w2b": w2b,
        "w18a": w18a, "w18b": w18b, "w28a": w28a, "w28b": w28b,
        "brow": brow,
        "ones1": np.ones((1, 128), np.float16),
        "ident": np.eye(128, dtype=np.float16),
        "smat": smat,
    }


def kernel(x, W0, b0, W1, b1, W2, b2):
    x = np.asarray(x)
    assert x.shape == (B, F0, D), x.shape
    nc = _get_nc()
    shared = _prep_weights(W0, b0, W1, b1, W2, b2)
    iu0, iu1 = np.triu_indices(F0)

    in_maps = []
    for c in range(NCORES):
        xc = x[c * BC:(c + 1) * BC]                      # [256, 39, 16]
        x0c = np.ascontiguousarray(
            xc.transpose(1, 0, 2).reshape(F0, N)).astype(np.float16)
        x0f32 = x0c.astype(np.float32)
        v0 = np.zeros((K0 * 128, N), np.float16)
        v0[0:NPAIR] = (x0f32[iu0] * x0f32[iu1]).astype(np.float16)
        in_maps.append({"x0": x0c, "v0": v0, **shared})

    res = run_bass_kernel_spmd(nc, in_maps, list(range(NCORES)))

    out = np.empty((B, 3 * C), dtype=np.float32)
    for c in range(NCORES):
        # outa cols: g*192 + l*64 + tl*16 + b16, tiles t = g*4+tl
        oa = res.results[c]["outa"]                      # [128, 768]
        ob = res.results[c]["outb"]                      # [72, 768]
        oa = oa.reshape(GA, 4, 3, GRP, BT).transpose(1, 3, 4, 2, 0)
        ob = ob.reshape(GB, 4, 3, GRP, BT).transpose(1, 3, 4, 2, 0)
        oc = np.concatenate(
            [oa.reshape(BC, 3, GA), ob.reshape(BC, 3, GB)], axis=2)
        out[c * BC:(c + 1) * BC] = oc.reshape(BC, 3 * C)
    return out


# revision 33
# speedup vs baseline: 1.1001x; 1.0001x over previous
"""CIN (Compressed Interaction Network) kernel for Trainium2, 8 NeuronCores.

Problem: x (2048, 39, 16) f32; 3 CIN layers with W_i (200, 39, prev):
    z[b,o,d] = sum_{f,g} W[o,f,g] * x0[b,f,d] * h[b,g,d] + bias[o]
    h' = relu(z);  output = sum_d concat([h1,h2,h3], ch) -> (2048, 600)

Strategy (data-parallel over batch, 8 cores, 256 batch rows each):
  Per core, columns n = (b_local, d), N = 256*16 = 4096, in 16 n-tiles of 256
  (two 128-column windows each).  Matmuls run in the z^T orientation:
  psum [128 n, 200 o] accumulates over the contraction (f, g); lhsT
  (stationary) = V slices [g, n-window]; rhs (moving) = weight slices
  [g, 200].  V_f = h (.) bcast(x0[f]) is split across engines by f-slot:
    slots 0..PF-1         fp8 direct on Pool (tensor_tensor)
    slots PF..PF+YF-1     fp16 on Vector (tensor_tensor, 2x mode),
                          converted to fp8 by the Scalar engine
    slots PF+YF..38       fp16 on Vector, consumed by fp16 matmuls
  fp8 slots feed fp8e4m3 DoubleRow matmuls (two f's per instruction at
  0.5 cycles/row).  Weights are pre-scaled by 64 so fp8 stays in e4m3's
  normal range; the relu epilogue on the Scalar engine rescales by 1/64.
  Bias enters as a K=1 ones-row matmul.  L0 uses the f<=g symmetry of
  x0*x0: 780 rows (7 K-chunks) with folded weights W0+W0^T.
  h^T [n, 200] is transposed by the PE into a PSUM bank; the Vector
  engine copies it to SBUF h tiles [g, n] for the next layer's V build.
  The d-sums run on the PE as tiny K=128 matmuls against a 0/1 selector
  [128, 8], accumulated in a PSUM bank that the Scalar engine flushes
  to SBUF every 4 tiles (DMA'd to DRAM at the end).  Tiles are emitted
  pairwise-interleaved, with the next pair's L0 layer emitted between
  the current pair's L1 and L2 so the PE fills build-phase bubbles and
  h0 of the next pair is ready before its V1 build; all build/copy/
  convert instructions are window-granular (128 columns) to keep the
  cross-engine dependency chain latency short.
"""
import numpy as np

import concourse.bacc as bacc
import concourse.mybir as mybir
import concourse.tile as tile
from concourse.bass_utils import run_bass_kernel_spmd

B, F0, D = 2048, 39, 16
C = 200                      # cross size per layer
NCORES = 8
BC = B // NCORES             # 256 batch rows per core
N = BC * D                   # 4096 columns per core
NT = 256                     # n-tile width
T = N // NT                  # 16 n-tiles
BT = NT // D                 # 16 batch rows per n-tile
NW = NT // 128               # 2 matmul windows per tile
K0 = 7                       # L0 symmetric K-chunks (780 rows padded to 896)
NPAIR = F0 * (F0 + 1) // 2   # 780
PF = 8                       # f-slots built fp8 directly on Pool
YF = 14                      # f-slots built fp16 on Vector, Act-converted
XF = F0 - PF - YF            # 17 f-slots kept fp16 end to end
NF8 = PF + YF                # 26 fp8 slots
NP8 = NF8 // 2               # 13 DoubleRow pairs
NV16 = YF + XF               # 28 f-slots built on Vector (fp16)
GA, GB = 128, C - 128        # g-split (h partition split 128 + 72)
SCALE = 64.0                 # weight pre-scale (power of 2)
GRP = 4                      # tiles per output-psum flush group
F16 = mybir.dt.float16
F8 = mybir.dt.float8e4
F32 = mybir.dt.float32


def _build_nc():
    nc = bacc.Bacc(None, target_bir_lowering=False)
    mult = mybir.AluOpType.mult
    relu = mybir.ActivationFunctionType.Relu

    x0_d = nc.dram_tensor("x0", [F0, N], F16, kind="ExternalInput")
    v0_d = nc.dram_tensor("v0", [K0 * 128, N], F16, kind="ExternalInput")
    w0_d = nc.dram_tensor("w0", [128, K0 * C], F16, kind="ExternalInput")
    w1a_d = nc.dram_tensor("w1a", [GA, XF * C], F16, kind="ExternalInput")
    w1b_d = nc.dram_tensor("w1b", [GB, XF * C], F16, kind="ExternalInput")
    w2a_d = nc.dram_tensor("w2a", [GA, XF * C], F16, kind="ExternalInput")
    w2b_d = nc.dram_tensor("w2b", [GB, XF * C], F16, kind="ExternalInput")
    w18a_d = nc.dram_tensor("w18a", [GA, NF8 * C], F8, kind="ExternalInput")
    w18b_d = nc.dram_tensor("w18b", [GB, NF8 * C], F8, kind="ExternalInput")
    w28a_d = nc.dram_tensor("w28a", [GA, NF8 * C], F8, kind="ExternalInput")
    w28b_d = nc.dram_tensor("w28b", [GB, NF8 * C], F8, kind="ExternalInput")
    brow_d = nc.dram_tensor("brow", [1, 3 * C], F16, kind="ExternalInput")
    ones_d = nc.dram_tensor("ones1", [1, 128], F16, kind="ExternalInput")
    id_d = nc.dram_tensor("ident", [128, 128], F16, kind="ExternalInput")
    smat_d = nc.dram_tensor("smat", [128, BT // NW], F16, kind="ExternalInput")
    outa_d = nc.dram_tensor("outa", [GA, 3 * N // D], F32, kind="ExternalOutput")
    outb_d = nc.dram_tensor("outb", [GB, 3 * N // D], F32, kind="ExternalOutput")

    with tile.TileContext(nc) as tc:
        with (
            tc.tile_pool(name="wp", bufs=1) as wp,
            tc.tile_pool(name="bc", bufs=2) as bcp,
            tc.tile_pool(name="hs", bufs=2) as hsp,
            tc.tile_pool(name="ht", bufs=6) as htp,
            tc.tile_pool(name="va", bufs=2) as vap,
            tc.tile_pool(name="ps", bufs=3, space="PSUM") as ps,
            tc.tile_pool(name="pt", bufs=3, space="PSUM") as pt,
            tc.tile_pool(name="op", bufs=2, space="PSUM") as opp,
        ):
            # --- static state -------------------------------------------------
            w0 = wp.tile([128, K0 * C], F16)
            nc.sync.dma_start(out=w0[:], in_=w0_d[:])
            brow = wp.tile([1, 3 * C], F16)
            nc.sync.dma_start(out=brow[:], in_=brow_d[:])
            ones1 = wp.tile([1, 128], F16)
            nc.sync.dma_start(out=ones1[:], in_=ones_d[:])
            ident = wp.tile([128, 128], F16)
            nc.sync.dma_start(out=ident[:], in_=id_d[:])
            smat = wp.tile([128, BT // NW], F16)
            nc.sync.dma_start(out=smat[:], in_=smat_d[:])
            outa_s = wp.tile([GA, 3 * N // D], F32)
            outb_s = wp.tile([GB, 3 * N // D], F32)

            def emit_v0(t):
                v0t = bcp.tile([128, K0 * NT], F16, tag="v0t")
                src = (v0_d[:].rearrange("(c p) n -> p c n", p=128)
                       [:, :, t * NT:(t + 1) * NT])
                nc.sync.dma_start(
                    out=v0t[:].rearrange("p (c n) -> p c n", n=NT), in_=src)
                return v0t

            def emit_xb(t, fchunk=13):
                xb = bcp.tile([128, F0 * NT], F16, tag="xb")
                for f0 in range(0, F0, fchunk):
                    f1 = min(f0 + fchunk, F0)
                    src = (x0_d[f0:f1, t * NT:(t + 1) * NT]
                           .unsqueeze(0).broadcast_to((128, f1 - f0, NT)))
                    nc.sync.dma_start(
                        out=xb[:, f0 * NT:f1 * NT]
                        .rearrange("p (f n) -> p f n", n=NT), in_=src)
                return xb, None

            def emit_build(xbp, ha, hb):
                # Window-granular build: every instruction covers one
                # 128-column window so the downstream matmul/conv chain
                # starts after ~1-2 us instead of ~4-8 us.
                #   va8/vb8 [*, NF8*NT] f8: slots 0..PF-1 Pool-direct,
                #     slots PF..NF8-1 Act-converted from va cols 0..YF-1
                #   va/vb [*, NV16*NT] f16: col j <-> f-slot PF+j
                xb, xbr = xbp
                va = vap.tile([GA, NV16 * NT], F16, tag="va")
                vb = vap.tile([GB, NV16 * NT], F16, tag="vb")
                va8 = vap.tile([GA, NF8 * NT], F8, tag="va8")
                vb8 = vap.tile([GB, NF8 * NT], F8, tag="vb8")
                v8w = va8[:].rearrange("p (f w n) -> p f w n", f=NF8, w=NW)
                v8bw = vb8[:].rearrange("p (f w n) -> p f w n", f=NF8, w=NW)
                vaw = va[:].rearrange("p (f w n) -> p f w n", f=NV16, w=NW)
                vbw = vb[:].rearrange("p (f w n) -> p f w n", f=NV16, w=NW)
                xbw = xb[:].rearrange("p (f w n) -> p f w n", f=F0, w=NW)
                for w in range(NW):
                    nc.gpsimd.tensor_tensor(
                        out=v8w[:, 0:PF, w],
                        in0=ha[:, w].unsqueeze(1).broadcast_to((GA, PF, 128)),
                        in1=xbw[0:GA, 0:PF, w], op=mult)
                    nc.gpsimd.tensor_tensor(
                        out=v8bw[:, 0:PF, w],
                        in0=hb[:, w].unsqueeze(1).broadcast_to((GB, PF, 128)),
                        in1=xbw[0:GB, 0:PF, w], op=mult)
                    nc.vector.tensor_tensor(
                        out=vaw[:, 0:YF, w],
                        in0=ha[:, w].unsqueeze(1).broadcast_to((GA, YF, 128)),
                        in1=xbw[0:GA, PF:PF + YF, w], op=mult)
                    nc.vector.tensor_tensor(
                        out=vbw[:, 0:YF, w],
                        in0=hb[:, w].unsqueeze(1).broadcast_to((GB, YF, 128)),
                        in1=xbw[0:GB, PF:PF + YF, w], op=mult)
                    nc.scalar.copy(out=v8w[:, PF:NF8, w],
                                   in_=vaw[:, 0:YF, w])
                    nc.scalar.copy(out=v8bw[:, PF:NF8, w],
                                   in_=vbw[:, 0:YF, w])
                    nc.vector.tensor_tensor(
                        out=vaw[:, YF:NV16, w],
                        in0=ha[:, w].unsqueeze(1).broadcast_to((GA, XF, 128)),
                        in1=xbw[0:GA, PF + YF:F0, w], op=mult)
                    nc.vector.tensor_tensor(
                        out=vbw[:, YF:NV16, w],
                        in0=hb[:, w].unsqueeze(1).broadcast_to((GB, XF, 128)),
                        in1=xbw[0:GB, PF + YF:F0, w], op=mult)
                return va, vb, va8, vb8

            def emit_l0_win(v0t, w, l):
                pz = ps.tile([128, C], F32, tag="pz")
                nc.tensor.matmul(pz[:], ones1[:],
                                 brow[:, l * C:(l + 1) * C],
                                 start=True, stop=False)
                v3 = v0t[:].rearrange("p (c n) -> p c n", n=NT)
                for c in range(K0):
                    nc.tensor.matmul(pz[:], v3[:, c, w * 128:(w + 1) * 128],
                                     w0[:, c * C:(c + 1) * C],
                                     start=False, stop=(c == K0 - 1))
                return pz

            def emit_l12_win(vs, wa, wb, w8a, w8b, w, l):
                va, vb, va8, vb8 = vs
                pz = ps.tile([128, C], F32, tag="pz")
                ws = slice(w * 128, (w + 1) * 128)
                nc.tensor.matmul(pz[:], ones1[:],
                                 brow[:, l * C:(l + 1) * C],
                                 start=True, stop=False)
                v3a = va[:].rearrange("p (f n) -> p f n", n=NT)
                v3b = vb[:].rearrange("p (f n) -> p f n", n=NT)
                for j in range(XF):
                    # fp16 f-slot PF+YF+j lives at va col YF+j
                    p = YF + j
                    nc.tensor.matmul(pz[:], v3a[:, p, ws],
                                     wa[:, j * C:(j + 1) * C],
                                     start=False, stop=False)
                    nc.tensor.matmul(pz[:], v3b[:, p, ws],
                                     wb[:, j * C:(j + 1) * C],
                                     start=False, stop=False)
                p3a = va8[:].rearrange("p (f n) -> p f n", n=NT)
                p3b = vb8[:].rearrange("p (f n) -> p f n", n=NT)
                w4a = w8a[:].rearrange("p (j t o) -> p j t o", t=2, o=C)
                w4b = w8b[:].rearrange("p (j t o) -> p j t o", t=2, o=C)
                for j in range(NP8):
                    nc.tensor.matmul(pz[:], p3a[:, 2 * j:2 * j + 2, ws],
                                     w4a[:, j], start=False, stop=False,
                                     perf_mode=mybir.MatmulPerfMode.DoubleRow)
                    nc.tensor.matmul(pz[:], p3b[:, 2 * j:2 * j + 2, ws],
                                     w4b[:, j], start=False, stop=(j == NP8 - 1),
                                     perf_mode=mybir.MatmulPerfMode.DoubleRow)
                return pz

            def emit_epi(pz, w, l, t, pout, hps=None, ha=None, hb=None,
                         hsb=None, do_copies=True):
                # relu (+1/64 rescale) -> h^T [128, 200]; d-sum on the PE;
                # for l<2: transpose into a PSUM bank, copy to SBUF h tiles
                hT = htp.tile([128, C], F16, tag="hT")
                nc.scalar.activation(hT[:], pz[:], relu, scale=1.0 / SCALE)
                if l < 2:
                    ws = slice(2 * w * 128, (2 * w + 1) * 128)
                    ws2 = slice((2 * w + 1) * 128, (2 * w + 2) * 128)
                    nc.tensor.transpose(hps[:, ws], hT[:, 0:GA], ident[:])
                    nc.tensor.transpose(hps[0:GB, ws2], hT[:, GA:C], ident[:])
                    if do_copies:
                        wf = slice(2 * w * 128, (2 * w + 2) * 128)
                        nc.vector.tensor_copy(out=hsb[:, wf],
                                              in_=hps[:, wf])
                off = l * 64 + (t % GRP) * BT + w * (BT // NW)
                nc.tensor.matmul(pout[:, off:off + BT // NW],
                                 hT[:, 0:GA], smat[:], start=True, stop=True)
                nc.tensor.matmul(pout[0:GB, 192 + off:192 + off + BT // NW],
                                 hT[:, GA:C], smat[:], start=True, stop=True)

            # --- weights ------------------------------------------------------
            v00, v01 = emit_v0(0), emit_v0(1)
            xb0, xb1 = emit_xb(0), emit_xb(1)
            w1a = wp.tile([GA, XF * C], F16)
            nc.sync.dma_start(out=w1a[:], in_=w1a_d[:])
            w1b = wp.tile([GB, XF * C], F16)
            nc.sync.dma_start(out=w1b[:], in_=w1b_d[:])
            w18a = wp.tile([GA, NF8 * C], F8)
            nc.sync.dma_start(out=w18a[:], in_=w18a_d[:])
            w18b = wp.tile([GB, NF8 * C], F8)
            nc.sync.dma_start(out=w18b[:], in_=w18b_d[:])
            w2a = wp.tile([GA, XF * C], F16)
            nc.sync.dma_start(out=w2a[:], in_=w2a_d[:])
            w2b = wp.tile([GB, XF * C], F16)
            nc.sync.dma_start(out=w2b[:], in_=w2b_d[:])
            w28a = wp.tile([GA, NF8 * C], F8)
            nc.sync.dma_start(out=w28a[:], in_=w28a_d[:])
            w28b = wp.tile([GB, NF8 * C], F8)
            nc.sync.dma_start(out=w28b[:], in_=w28b_d[:])

            # --- pipeline: rotated so L0(p+1) overlaps V2(p)/L2(p) -----------
            def alloc_h(tp, l):
                hts = []
                for k in range(2):
                    hps = pt.tile([128, 2 * NT], F16, tag="hps",
                                  name=f"hps{l}_{tp}_{k}")
                    hsb = hsp.tile([GA, 2 * NT], F16, tag=f"h{l}s{k}",
                                   name=f"h{l}s{k}_{tp}")
                    # window-major: [w0-a 0:128, w0-b 128:256, w1-a ...]
                    ha = hsb[:].rearrange("p (w t n) -> p w t n",
                                          w=NW, t=2)[:, :, 0, :]
                    hb = (hsb[0:GB].rearrange("p (w t n) -> p w t n",
                                              w=NW, t=2)[:, :, 1, :])
                    hts.append((hps, ha, hb, hsb))
                return hts

            def emit_l0_pair(tp, v0s, pout, do_copies=True, copy_eng=None):
                hts = alloc_h(tp, 0)
                for w in range(NW):
                    for k in range(2):
                        pz = emit_l0_win(v0s[k], w, 0)
                        emit_epi(pz, w, 0, tp + k, pout, *hts[k],
                                 do_copies=do_copies)
                    if copy_eng is not None:
                        for k in range(2):
                            emit_h_copies(hts[k], eng=copy_eng, w=w)
                return hts

            def emit_h_copies(hts_k, eng="dve", w=None):
                hps, ha, hb, hsb = hts_k
                sl = (slice(0, 2 * NT) if w is None
                      else slice(2 * w * 128, (2 * w + 2) * 128))
                if eng == "dve":
                    nc.vector.tensor_copy(out=hsb[:, sl], in_=hps[:, sl])
                else:
                    nc.scalar.copy(out=hsb[:, sl], in_=hps[:, sl])

            # prologue: L0 of pair 0
            pout = opp.tile([128, 2 * 192], F32, tag="pout")
            h0s = emit_l0_pair(0, (v00, v01), pout, do_copies=False,
                               copy_eng="act")
            xbs = (xb0, xb1)

            for tp in range(0, T, 2):
                t0, t1 = tp, tp + 1
                # prefetch pair p+1 inputs early
                if tp + 2 < T:
                    v0n = (emit_v0(tp + 2), emit_v0(tp + 3))
                    xbn = (emit_xb(tp + 2), emit_xb(tp + 3))
                # V1 builds + L1 windows
                h1s = alloc_h(tp, 1)
                vss = [emit_build(xbs[k], h0s[k][1], h0s[k][2])
                       for k in range(2)]
                for w in range(NW):
                    for k, t in ((0, t0), (1, t1)):
                        pz = emit_l12_win(vss[k], w1a, w1b, w18a, w18b, w, 1)
                        emit_epi(pz, w, 1, t, pout, *h1s[k],
                                 do_copies=False)
                    for k in range(2):
                        emit_h_copies(h1s[k], eng="act", w=w)
                # L0 of pair p+1 runs on the PE while V2 builds; its h
                # copies interleave between the V2-A and V2-B build chunks
                pout_cur = pout
                h0n = None
                if tp + 2 < T:
                    if (tp + 2) % GRP == 0:
                        pout = opp.tile([128, 2 * 192], F32, tag="pout")
                    h0n = emit_l0_pair(tp + 2, v0n, pout, do_copies=False)
                    for k in range(2):
                        emit_h_copies(h0n[k], eng="act")
                vss = [emit_build(xbs[k], h1s[k][1], h1s[k][2])
                       for k in range(2)]
                if tp + 2 < T:
                    h0s = h0n
                    xbs = xbn
                # L2 windows
                for w in range(NW):
                    for k, t in ((0, t0), (1, t1)):
                        pz = emit_l12_win(vss[k], w2a, w2b, w28a, w28b, w, 2)
                        emit_epi(pz, w, 2, t, pout_cur)
                if tp % GRP == GRP - 2:
                    g = tp // GRP
                    nc.scalar.copy(out=outa_s[:, g * 192:(g + 1) * 192],
                                   in_=pout_cur[:, 0:192])
                    nc.scalar.copy(out=outb_s[:, g * 192:(g + 1) * 192],
                                   in_=pout_cur[0:GB, 192:384])

            nc.sync.dma_start(out=outa_d[:], in_=outa_s[:])
            nc.sync.dma_start(out=outb_d[:], in_=outb_s[:])

    nc.compile()
    return nc


_NC_CACHE = None


def _get_nc():
    global _NC_CACHE
    if _NC_CACHE is None:
        _NC_CACHE = _build_nc()
    return _NC_CACHE


def _q8(x):
    import ml_dtypes
    return np.asarray(x, np.float32).astype(ml_dtypes.float8_e4m3fn)


def _prep_weights(W0, b0, W1, b1, W2, b2):
    # L0: symmetric fold.  W0eff[o, (f,g)] = W0[o,f,g]+W0[o,g,f] (f<g),
    # W0[o,f,f] on the diagonal; pairs in triu order, padded 780 -> 896.
    W0 = np.asarray(W0, np.float32)
    iu0, iu1 = np.triu_indices(F0)
    Wsym = W0 + W0.transpose(0, 2, 1)
    dd = np.arange(F0)
    Wsym[:, dd, dd] = W0[:, dd, dd]
    w0f = np.zeros((K0 * 128, C), np.float32)
    w0f[0:NPAIR] = Wsym[:, iu0, iu1].T * SCALE
    w0 = np.ascontiguousarray(
        w0f.reshape(K0, 128, C).transpose(1, 0, 2).reshape(128, K0 * C)
    ).astype(np.float16)

    def lay(W):
        # Wt[g, f, o] = SCALE*W[o, f, g]; slot order = f order
        Wt = np.asarray(W, np.float32).transpose(2, 1, 0) * SCALE
        wa = np.ascontiguousarray(Wt[0:GA, NF8:].reshape(GA, XF * C)
                                  ).astype(np.float16)
        wb = np.ascontiguousarray(Wt[GA:C, NF8:].reshape(GB, XF * C)
                                  ).astype(np.float16)
        w8a = _q8(np.ascontiguousarray(Wt[0:GA, 0:NF8].reshape(GA, NF8 * C)))
        w8b = _q8(np.ascontiguousarray(Wt[GA:C, 0:NF8].reshape(GB, NF8 * C)))
        return wa, wb, w8a, w8b

    w1a, w1b, w18a, w18b = lay(W1)
    w2a, w2b, w28a, w28b = lay(W2)
    brow = np.zeros((1, 3 * C), np.float16)
    for l, b in enumerate((b0, b1, b2)):
        brow[0, l * C:(l + 1) * C] = (np.asarray(b, np.float32) * SCALE
                                      ).astype(np.float16)
    smat = np.zeros((128, BT // NW), np.float16)
    smat[np.arange(128), np.arange(128) // D] = 1.0
    return {
        "w0": w0, "w1a": w1a, "w1b": w1b, "w2a": w2a, "w2b": w2b,
        "w18a": w18a, "w18b": w18b, "w28a": w28a, "w28b": w28b,
        "brow": brow,
        "ones1": np.ones((1, 128), np.float16),
        "ident": np.eye(128, dtype=np.float16),
        "smat": smat,
    }


def kernel(x, W0, b0, W1, b1, W2, b2):
    x = np.asarray(x)
    assert x.shape == (B, F0, D), x.shape
    nc = _get_nc()
    shared = _prep_weights(W0, b0, W1, b1, W2, b2)
    iu0, iu1 = np.triu_indices(F0)

    in_maps = []
    for c in range(NCORES):
        xc = x[c * BC:(c + 1) * BC]                      # [256, 39, 16]
        x0c = np.ascontiguousarray(
            xc.transpose(1, 0, 2).reshape(F0, N)).astype(np.float16)
        x0f32 = x0c.astype(np.float32)
        v0 = np.zeros((K0 * 128, N), np.float16)
        v0[0:NPAIR] = (x0f32[iu0] * x0f32[iu1]).astype(np.float16)
        in_maps.append({"x0": x0c, "v0": v0, **shared})

    res = run_bass_kernel_spmd(nc, in_maps, list(range(NCORES)))

    out = np.empty((B, 3 * C), dtype=np.float32)
    for c in range(NCORES):
        # outa cols: g*192 + l*64 + tl*16 + b16, tiles t = g*4+tl
        oa = res.results[c]["outa"]                      # [128, 768]
        ob = res.results[c]["outb"]                      # [72, 768]
        oa = oa.reshape(GA, 4, 3, GRP, BT).transpose(1, 3, 4, 2, 0)
        ob = ob.reshape(GB, 4, 3, GRP, BT).transpose(1, 3, 4, 2, 0)
        oc = np.concatenate(
            [oa.reshape(BC, 3, GA), ob.reshape(BC, 3, GB)], axis=2)
        out[c * BC:(c + 1) * BC] = oc.reshape(BC, 3 * C)
    return out
